# revision 21
# baseline (speedup 1.0000x reference)
"""Trainium2 Bass kernel: AtomEmbeddingAndSumLastLayer (segment_reduce).

Computes: out = normalize(relu(segment_sum(relu(x @ W.T + b), batch)))
  x [1M, 118] f32, W [64, 118], b [64], batch [1M] sorted int in [0, 4096).

Strategy (8 NeuronCores, no collectives needed):
  - Atoms are cut at segment-aligned boundaries on the host so core c owns
    exactly segments [512c, 512(c+1)); per-core outputs concatenate.
  - Host pre-transposes x to xT [119, A] fp8-e4m3 with a ones-row at 118
    (folds the bias into the matmul); atoms are grouped into 4 "superwindows"
    of 128 segments, each made of 4 windows of 32 segments whose 128-atom
    tiles are interleaved quad-wise.
  - Device, per 128-atom tile:
      h_psum[128, 64] = xT_tile.T @ WT            (TensorE, fp8 lhsT + FWL)
      h_sb = relu(h_psum) -> bf16                 (split ScalarE / VectorE)
      s_psum[32q:32q+32, 64] += oh_t.T @ h_sb     (TensorE col-group q)
    One-hot oh is built g-major per superwindow ([128, g*sw_tiles + m] =
    (seg[m] == g)) by 32 tensor_scalar(is_equal, scalar=g) ops split between
    VectorE (4x_2p mode) and GpSimd; the seg-matmul reads it through a
    strided weights AP.  PE is pre-warmed (HAM clock) with dummy matmuls
    during the initial x DMA.
    Epilogue per superwindow on [128, 64]: rowwise max, recip, scale, DMA.
"""

import os
import sys
import numpy as np

sys.path.insert(0, "/opt/trn_rl_repo")

import ml_dtypes  # noqa: E402

N_ATOMS = 1_000_000
D_IN = 118
K_DIM = 119  # 118 features + ones-row (bias) at 118
ONES_ROW = D_IN
D_OUT = 64
NUM_SEG = 4096
N_CORES = 8
SEGS_PER_CORE = NUM_SEG // N_CORES  # 512
G_W = 32  # segments per window (one PE col-group)
QUAD = 4  # windows per superwindow (= PE col-groups used)
SUPER = SEGS_PER_CORE // (G_W * QUAD)  # 4 superwindows per core
P = 128
CHUNK = 16  # tiles per compute chunk (= 4 quads; 16*64 f32 = 2 psum banks)
RSPLIT = 11  # tiles per chunk relu'd on ScalarE (rest on VectorE)
WARM_MM = 16  # dummy matmuls to warm the PE HAM clock during initial DMA
XBUFS = 32  # x-piece ring depth (4 KB/partition pieces; all coexist)
HBUFS = 4
OHBUFS = 3
PAD_ID = 200.0  # local seg id for padding atoms; never matches any g

BF16 = ml_dtypes.bfloat16
FP8 = ml_dtypes.float8_e4m3

_CACHE = {}


def _build_graph(t_q: int, postprocess: bool = True):
    """Build the SPMD Bass graph for one core.

    t_q = padded tiles per window (multiple of QUAD); each superwindow has
    QUAD * t_q interleaved tiles.
    """
    import concourse.bass as bass
    import concourse.tile as tile
    from concourse import mybir
    from contextlib import ExitStack

    sw_tiles = QUAD * t_q  # tiles per superwindow
    n_tiles = SUPER * sw_tiles
    a_cols = n_tiles * P

    nc = bass.Bass(target_bir_lowering=False)

    xt = nc.declare_dram_parameter("xt", [K_DIM, a_cols], mybir.dt.float8e4, False)
    # seg ids duplicated pairwise on the host: seg2[p, 2t+j] = seg[p, t].
    # The duplication gives every operand of the one-hot is_equal a packed
    # 2-element innermost AP dim, unlocking the DVE 2x_1p mode.
    seg = nc.declare_dram_parameter("seg", [P, 2 * n_tiles], mybir.dt.bfloat16, False)
    wt = nc.declare_dram_parameter("wt", [K_DIM, D_OUT], mybir.dt.bfloat16, False)
    iota = nc.declare_dram_parameter("iota", [P, G_W], mybir.dt.bfloat16, False)
    out = nc.declare_dram_parameter(
        "out", [SEGS_PER_CORE, D_OUT], mybir.dt.float32, True
    )

    with ExitStack() as ctx:
        tc = ctx.enter_context(tile.TileContext(nc))
        consts = ctx.enter_context(tc.tile_pool(name="consts", bufs=1))
        xpool = ctx.enter_context(tc.tile_pool(name="xp", bufs=XBUFS))
        hpool_s = ctx.enter_context(tc.tile_pool(name="hps", bufs=HBUFS))
        hpool_v = ctx.enter_context(tc.tile_pool(name="hpv", bufs=HBUFS))
        ohpool = ctx.enter_context(tc.tile_pool(name="ohp", bufs=OHBUFS))
        psum_h = ctx.enter_context(tc.tile_pool(name="psh", bufs=3, space="PSUM"))
        psum_s = ctx.enter_context(tc.tile_pool(name="pss", bufs=2, space="PSUM"))
        epi = ctx.enter_context(tc.tile_pool(name="epi", bufs=2))

        # issue ALL x-piece DMAs first, on the otherwise-idle GpSimd queue:
        # the kernel is DMA-delivery-bound (~170 GB/s per-core HBM share),
        # so x must stream continuously from t=0 with nothing ahead of it.
        # Small pieces give a fast start and a short compute tail.
        N_PIECES = 8  # per superwindow
        piece = sw_tiles * P // N_PIECES
        x_pieces_sw = []
        for sw in range(SUPER):
            base_t = sw * sw_tiles
            x_pieces = []
            for pi in range(N_PIECES):
                xp_t = xpool.tile([K_DIM, piece], mybir.dt.float8e4, tag="xp8")
                p0 = base_t * P + pi * piece
                # alternate between two DMA queues: a single queue's
                # descriptor feed tops out well below the DMA engines'
                # aggregate bandwidth
                eng = nc.gpsimd if (sw * N_PIECES + pi) % 2 == 0 else nc.sync
                eng.dma_start(out=xp_t[:], in_=xt[:, p0 : p0 + piece])
                x_pieces.append(xp_t)
            x_pieces_sw.append((x_pieces, piece))

        wt_sb = consts.tile([K_DIM, D_OUT], mybir.dt.bfloat16)
        nc.sync.dma_start(out=wt_sb[:], in_=wt[:, :])
        iota_sb = consts.tile([P, G_W], mybir.dt.bfloat16)
        nc.sync.dma_start(out=iota_sb[:], in_=iota[:, :])
        seg_sb = consts.tile([P, 2 * n_tiles], mybir.dt.bfloat16)
        nc.sync.dma_start(out=seg_sb[:], in_=seg[:, :])

        # "touch" the consts on VectorE once so later ops don't each carry
        # multiple DMA-lane semaphore waits (walrus wait-slot limit).
        dummy_a = consts.tile([P, 1], mybir.dt.bfloat16)
        nc.vector.tensor_copy(out=dummy_a[:], in_=iota_sb[:, :1])
        dummy_b = consts.tile([P, 1], mybir.dt.bfloat16)
        nc.vector.tensor_copy(out=dummy_b[:], in_=seg_sb[:, :1])
        dummy_c = consts.tile([K_DIM, 1], mybir.dt.bfloat16)
        nc.vector.tensor_copy(out=dummy_c[:], in_=wt_sb[:, :1])
        zeros_sb = consts.tile([P, P], mybir.dt.bfloat16)
        nc.vector.memset(zeros_sb[:], 0.0)
        # prewarm ScalarE's activation table during the initial x DMA
        dummy_d = consts.tile([P, 1], mybir.dt.bfloat16)
        nc.scalar.activation(
            out=dummy_d[:], in_=dummy_b[:],
            func=mybir.ActivationFunctionType.Relu,
        )

        # warm the PE HAM clock gate during the initial DMA fill: ~16
        # zero matmuls (N=512) keep the array busy ~3.4us so real work
        # starts at 2.4 GHz.  Depends only on seg/wt DMA + zeros memset.
        warm_ps = psum_h.tile([P, CHUNK * D_OUT], mybir.dt.float32, tag="hp")
        for w in range(WARM_MM):
            nc.tensor.matmul(
                out=warm_ps[:, :512],
                lhsT=zeros_sb[:K_DIM, :],
                rhs=seg_sb[:K_DIM, :512],
                start=True,
                stop=True,
                skip_group_check=True,
            )

        GH, GL = G_W // 2, 2

        def emit_oh(sw, oh_t, part, n_parts):
            """One slice of superwindow sw's one-hot (t-major layout):
            oh[p, m*G_W + g] = (seg[p, base+m] == g).  Expressed over dims
            (m, g_hi, g_lo) so every operand's innermost AP dim is a packed
            pair -> DVE 2x_1p mode.  `part` slices the tile range."""
            base_t = sw * sw_tiles
            m0 = part * sw_tiles // n_parts
            m1 = (part + 1) * sw_tiles // n_parts
            nm = m1 - m0
            o = oh_t[:, (m0 * G_W) : (m1 * G_W)]
            out_ap = bass.AP(
                tensor=o.tensor, offset=o.offset,
                ap=[o.ap[0], [G_W, nm], [GL, GH], [1, GL]],
            )
            iap = iota_sb[:]
            in0 = bass.AP(
                tensor=iap.tensor, offset=iap.offset,
                ap=[iap.ap[0], [0, nm], [GL, GH], [1, GL]],
            )
            s2 = seg_sb[:, 2 * (base_t + m0) : 2 * (base_t + m1)]
            in1 = bass.AP(
                tensor=s2.tensor, offset=s2.offset,
                ap=[s2.ap[0], [2, nm], [0, GH], [1, GL]],
            )
            nc.vector.tensor_tensor(
                out=out_ap, in0=in0, in1=in1, op=mybir.AluOpType.is_equal,
            )

        oh_tiles = {}

        def new_oh_tile(sw):
            t = ohpool.tile([P, G_W * sw_tiles], mybir.dt.bfloat16)
            oh_tiles[sw] = t
            return t

        # superwindow 0's one-hot upfront (runs during initial x DMA)
        oh0 = new_oh_tile(0)
        for part in range(2):
            emit_oh(0, oh0, part, 2)

        # Software-pipelined chunk loop: h-matmuls are emitted LOOKAHEAD
        # chunks ahead of the relu + seg-matmuls, so the in-order PE queue
        # always holds ready h work while a chunk's relu completes.  Without
        # this the PE idles each chunk (seg-MMs head-block the next h-MMs),
        # its duty cycle drops below the HAM busy threshold, and the whole
        # kernel runs at the cold 1.2 GHz clock.
        LOOKAHEAD = 2
        n_chunks = sw_tiles // CHUNK
        chunks = [(sw, chv) for sw in range(SUPER) for chv in range(n_chunks)]
        n_total = len(chunks)
        h_ctx = {}
        sw_state = {}

        def emit_h(ci):
            sw, chv = chunks[ci]
            x_pieces, piece = x_pieces_sw[sw]
            h_ps = psum_h.tile([P, CHUNK * D_OUT], mybir.dt.float32, tag="hp")
            for i in range(CHUNK):
                t = chv * CHUNK + i
                pi = (t * P) // piece
                toff = pi * piece
                nc.tensor.matmul(
                    out=h_ps[:, i * D_OUT : (i + 1) * D_OUT],
                    lhsT=x_pieces[pi][:, t * P - toff : (t + 1) * P - toff],
                    rhs=wt_sb[:],
                    start=True,
                    stop=True,
                )
            h_ctx[ci] = h_ps

        for ci in range(n_total + LOOKAHEAD):
            if ci < n_total:
                emit_h(ci)
            j = ci - LOOKAHEAD
            if j < 0:
                continue
            sw, chv = chunks[j]
            if chv == 0:
                s_ps = psum_s.tile([P, D_OUT], mybir.dt.float32)
                # open the accumulation group over the whole bank with a
                # zero matmul (clears has_written for all 128 partitions);
                # the col-group seg-matmuls accumulate with start=False
                nc.tensor.matmul(
                    out=s_ps[:],
                    lhsT=zeros_sb[:K_DIM, :],
                    rhs=wt_sb[:],
                    start=True,
                    stop=False,
                    skip_group_check=True,
                )
                oh_win = oh_tiles.pop(sw)
                oh_next = new_oh_tile(sw + 1) if sw + 1 < SUPER else None
                sw_state[sw] = (s_ps, oh_win, oh_next)
            s_ps, oh_win, oh_next = sw_state[sw]
            h_ps = h_ctx.pop(j)
            h_s = hpool_s.tile([P, RSPLIT * D_OUT], mybir.dt.bfloat16)
            nc.scalar.activation(
                out=h_s[:],
                in_=h_ps[:, : RSPLIT * D_OUT],
                func=mybir.ActivationFunctionType.Relu,
            )
            h_v = hpool_v.tile([P, (CHUNK - RSPLIT) * D_OUT],
                               mybir.dt.bfloat16)
            nc.vector.tensor_scalar_max(
                out=h_v[:], in0=h_ps[:, RSPLIT * D_OUT :], scalar1=0.0
            )
            if oh_next is not None:
                emit_oh(sw + 1, oh_next, chv, n_chunks)
            # seg-matmuls: window q of the quad accumulates on PE
            # col-group q into psum partitions [32q, 32q+32)
            for i in range(CHUNK):
                t = chv * CHUNK + i
                q = i % QUAD
                if i < RSPLIT:
                    rhs = h_s[:, i * D_OUT : (i + 1) * D_OUT]
                else:
                    rhs = h_v[:, (i - RSPLIT) * D_OUT : (i - RSPLIT + 1) * D_OUT]
                nc.tensor.matmul(
                    out=s_ps[G_W * q : G_W * (q + 1), :],
                    lhsT=oh_win[:, t * G_W : (t + 1) * G_W],
                    rhs=rhs,
                    start=False,
                    stop=(chv == n_chunks - 1 and i == CHUNK - 1),
                    tile_position=(0, G_W * q),
                    skip_group_check=True,
                )
            if chv == n_chunks - 1:
                # epilogue: max-normalize the superwindow's 128 segment rows
                s_sb = epi.tile([P, D_OUT], mybir.dt.float32)
                nc.vector.tensor_copy(out=s_sb[:], in_=s_ps[:])
                mx = epi.tile([P, 1], mybir.dt.float32)
                nc.vector.tensor_reduce(
                    out=mx[:], in_=s_sb[:], axis=mybir.AxisListType.X,
                    op=mybir.AluOpType.max,
                )
                rc = epi.tile([P, 1], mybir.dt.float32)
                nc.vector.reciprocal(out=rc[:], in_=mx[:])
                o_sb = epi.tile([P, D_OUT], mybir.dt.float32)
                nc.vector.tensor_scalar_mul(
                    out=o_sb[:], in0=s_sb[:], scalar1=rc[:]
                )
                nc.sync.dma_start(
                    out=out[sw * P : (sw + 1) * P, :], in_=o_sb[:]
                )

    if postprocess:
        _split_multi_waits(nc)
    return nc


def _split_multi_waits(nc):
    """walrus allows a single embedded sync wait per compute instruction.
    Move extra waits onto same-engine NoOps inserted just before."""
    from concourse import mybir

    n = 0
    for f in nc.m.functions:
        for blk in f.blocks:
            new_insts = []
            for inst in blk.instructions:
                si = getattr(inst, "sync_info", None)
                if si is not None and si.on_wait and len(si.on_wait) > 1:
                    extras, keep = si.on_wait[:-1], si.on_wait[-1:]
                    for wsub in extras:
                        nop = mybir.InstNoOp(
                            name=f"{inst.name}_waitnop{n}",
                            sync_info=mybir.SyncInfo(on_wait=[wsub], on_update=[]),
                            bass_nofuse=True,
                            engine=inst.engine,
                        )
                        n += 1
                        new_insts.append(nop)
                    si.on_wait = keep
                new_insts.append(inst)
            blk.instructions[:] = new_insts


def _prepare_inputs(x, w_mat, b, batch):
    """Host-side sharding/layout. Returns (in_maps, t_q)."""
    x = np.asarray(x, dtype=np.float32)
    w_mat = np.asarray(w_mat, dtype=np.float32)
    b = np.asarray(b, dtype=np.float32)
    batch = np.asarray(batch).astype(np.int64)

    # window boundaries: window j (global, 32 segs) holds atoms [wb[j], wb[j+1])
    wb = np.searchsorted(batch, np.arange(0, NUM_SEG + 1, G_W))
    counts = np.diff(wb)
    t_q = int(np.ceil(counts.max() / P))
    t_q = ((t_q + QUAD - 1) // QUAD) * QUAD  # multiple of QUAD

    sw_tiles = QUAD * t_q
    n_tiles = SUPER * sw_tiles
    a_cols = n_tiles * P

    wt = np.zeros((K_DIM, D_OUT), dtype=BF16)
    wt[:D_IN] = w_mat.T.astype(BF16)
    wt[ONES_ROW] = b.astype(BF16)
    iota = np.broadcast_to(
        np.arange(G_W, dtype=np.float32), (P, G_W)
    ).astype(BF16)

    xb = x.astype(FP8)
    n_win_per_core = SEGS_PER_CORE // G_W  # 16
    in_maps = []
    for c in range(N_CORES):
        xt_c = np.zeros((K_DIM, a_cols), dtype=FP8)
        seg_c = np.full((n_tiles, P), PAD_ID, dtype=np.float32)
        for sw in range(SUPER):
            for q in range(QUAD):
                gw = c * n_win_per_core + sw * QUAD + q  # global window id
                a0, a1 = wb[gw], wb[gw + 1]
                cnt = a1 - a0
                loc = (batch[a0:a1] - gw * G_W).astype(np.float32)
                # tile k of this window sits at interleaved slot (k*QUAD + q)
                for k in range((cnt + P - 1) // P):
                    m = sw * sw_tiles + k * QUAD + q  # global tile index
                    s0, s1 = k * P, min((k + 1) * P, cnt)
                    nseg = s1 - s0
                    col0 = m * P
                    xt_c[:D_IN, col0 : col0 + nseg] = xb[a0 + s0 : a0 + s1].T
                    xt_c[ONES_ROW, col0 : col0 + nseg] = 1.0
                    seg_c[m, :nseg] = loc[s0:s1]
        seg_c = np.ascontiguousarray(
            np.repeat(seg_c.T, 2, axis=1)
        ).astype(BF16)
        in_maps.append({"xt": xt_c, "seg": seg_c, "wt": wt, "iota": iota})
    return in_maps, t_q


def _install_ntff_hook_shim():
    """The trimmed container's antenv lacks axon_hooks; recreate it so
    run_bass_kernel_spmd(trace=True) can profile via the axon .so."""
    import types

    if "antenv.axon_hooks" in sys.modules:
        return
    try:
        from trn_agent_boot.trn_boot import _ntff_profile_via_ctypes

        hook = _ntff_profile_via_ctypes("/opt/axon/libaxon_pjrt.so")
    except Exception:
        hook = None
    mod = types.ModuleType("antenv.axon_hooks")
    mod._hook = hook
    mod.get_axon_ntff_profile_hook = lambda: mod._hook
    mod.set_axon_ntff_profile_hook = lambda h: setattr(mod, "_hook", h)
    sys.modules["antenv.axon_hooks"] = mod


def kernel(x, W, b, batch, num_segments):
    from concourse.bass_utils import run_bass_kernel_spmd

    assert int(num_segments) == NUM_SEG
    in_maps, t_q = _prepare_inputs(x, W, b, batch)

    key = (t_q, G_W, QUAD, CHUNK, RSPLIT, XBUFS, HBUFS, OHBUFS)
    if key not in _CACHE:
        _CACHE[key] = _build_graph(t_q)
    nc = _CACHE[key]

    trace = bool(int(os.environ.get("KERNEL_TRACE", "0")))
    if trace:
        _install_ntff_hook_shim()
    res = run_bass_kernel_spmd(
        nc, in_maps, core_ids=list(range(N_CORES)), trace=trace
    )
    kernel.last_result = res
    out = np.concatenate([r["out"] for r in res.results], axis=0)
    return out.astype(np.float32)


kernel.last_result = None


# revision 22
# speedup vs baseline: 1.4013x; 1.4013x over previous
"""Trainium2 Bass kernel: AtomEmbeddingAndSumLastLayer (segment_reduce).

Computes: out = normalize(relu(segment_sum(relu(x @ W.T + b), batch)))
  x [1M, 118] f32, W [64, 118], b [64], batch [1M] sorted int in [0, 4096).

Strategy (8 NeuronCores, no collectives needed):
  - Atoms are cut at segment-aligned boundaries on the host so core c owns
    exactly segments [512c, 512(c+1)); per-core outputs concatenate.
  - Host pre-transposes x to xT [119, A] fp8-e4m3 with a ones-row at 118
    (folds the bias into the matmul); atoms are grouped into 4 "superwindows"
    of 128 segments, each made of 4 windows of 32 segments whose 128-atom
    tiles are interleaved quad-wise.
  - Device, per 128-atom tile:
      h_psum[128, 64] = xT_tile.T @ WT            (TensorE, fp8 lhsT + FWL)
      h_sb = relu(h_psum) -> bf16                 (split ScalarE / VectorE)
      s_psum[32q:32q+32, 64] += oh_t.T @ h_sb     (TensorE col-group q)
    One-hot oh is built g-major per superwindow ([128, g*sw_tiles + m] =
    (seg[m] == g)) by 32 tensor_scalar(is_equal, scalar=g) ops split between
    VectorE (4x_2p mode) and GpSimd; the seg-matmul reads it through a
    strided weights AP.  PE is pre-warmed (HAM clock) with dummy matmuls
    during the initial x DMA.
    Epilogue per superwindow on [128, 64]: rowwise max, recip, scale, DMA.
"""

import os
import sys
import numpy as np

sys.path.insert(0, "/opt/trn_rl_repo")

import ml_dtypes  # noqa: E402

N_ATOMS = 1_000_000
D_IN = 118
K_DIM = 119  # 118 features + ones-row (bias) at 118
ONES_ROW = D_IN
D_OUT = 64
NUM_SEG = 4096
N_CORES = 8
SEGS_PER_CORE = NUM_SEG // N_CORES  # 512
G_W = 32  # segments per window (one PE col-group)
QUAD = 4  # windows per superwindow (= PE col-groups used)
SUPER = SEGS_PER_CORE // (G_W * QUAD)  # 4 superwindows per core
P = 128
CHUNK = 16  # tiles per compute chunk (= 4 quads; 16*64 f32 = 2 psum banks)
RSPLIT = 11  # tiles per chunk relu'd on ScalarE (rest on VectorE)
WARM_MM = 16  # dummy matmuls to warm the PE HAM clock during initial DMA
XBUFS = 32  # x-piece ring depth (4 KB/partition pieces; all coexist)
HBUFS = 4
OHBUFS = 3
PAD_ID = 200.0  # local seg id for padding atoms; never matches any g

BF16 = ml_dtypes.bfloat16
FP8 = ml_dtypes.float8_e4m3

_CACHE = {}


def _build_graph(t_q: int, postprocess: bool = True):
    """Build the SPMD Bass graph for one core.

    t_q = padded tiles per window (multiple of QUAD); each superwindow has
    QUAD * t_q interleaved tiles.
    """
    import concourse.bass as bass
    import concourse.tile as tile
    from concourse import mybir
    from contextlib import ExitStack

    sw_tiles = QUAD * t_q  # tiles per superwindow
    n_tiles = SUPER * sw_tiles
    a_cols = n_tiles * P

    nc = bass.Bass(target_bir_lowering=False)

    xt = nc.declare_dram_parameter("xt", [K_DIM, a_cols], mybir.dt.float8e4, False)
    # seg ids duplicated pairwise on the host: seg2[p, 2t+j] = seg[p, t].
    # The duplication gives every operand of the one-hot is_equal a packed
    # 2-element innermost AP dim, unlocking the DVE 2x_1p mode.
    seg = nc.declare_dram_parameter("seg", [P, 2 * n_tiles], mybir.dt.bfloat16, False)
    wt = nc.declare_dram_parameter("wt", [K_DIM, D_OUT], mybir.dt.bfloat16, False)
    iota = nc.declare_dram_parameter("iota", [P, G_W], mybir.dt.bfloat16, False)
    out = nc.declare_dram_parameter(
        "out", [SEGS_PER_CORE, D_OUT], mybir.dt.float32, True
    )

    with ExitStack() as ctx:
        tc = ctx.enter_context(tile.TileContext(nc))
        consts = ctx.enter_context(tc.tile_pool(name="consts", bufs=1))
        xpool = ctx.enter_context(tc.tile_pool(name="xp", bufs=XBUFS))
        hpool_s = ctx.enter_context(tc.tile_pool(name="hps", bufs=HBUFS))
        hpool_v = ctx.enter_context(tc.tile_pool(name="hpv", bufs=HBUFS))
        ohpool = ctx.enter_context(tc.tile_pool(name="ohp", bufs=OHBUFS))
        psum_h = ctx.enter_context(tc.tile_pool(name="psh", bufs=3, space="PSUM"))
        psum_s = ctx.enter_context(tc.tile_pool(name="pss", bufs=2, space="PSUM"))
        epi = ctx.enter_context(tc.tile_pool(name="epi", bufs=2))

        # issue ALL x-piece DMAs first, on the otherwise-idle GpSimd queue:
        # the kernel is DMA-delivery-bound (~170 GB/s per-core HBM share),
        # so x must stream continuously from t=0 with nothing ahead of it.
        # Small pieces give a fast start and a short compute tail.
        N_PIECES = 8  # per superwindow
        piece = sw_tiles * P // N_PIECES
        x_pieces_sw = []
        for sw in range(SUPER):
            base_t = sw * sw_tiles
            x_pieces = []
            for pi in range(N_PIECES):
                xp_t = xpool.tile([K_DIM, piece], mybir.dt.float8e4, tag="xp8")
                p0 = base_t * P + pi * piece
                # single queue => strictly sequential piece delivery
                # (delivery rate is HBM-share-bound either way; splitting
                # across queues reorders arrivals and stalls compute)
                nc.gpsimd.dma_start(out=xp_t[:], in_=xt[:, p0 : p0 + piece])
                x_pieces.append(xp_t)
            x_pieces_sw.append((x_pieces, piece))

        wt_sb = consts.tile([K_DIM, D_OUT], mybir.dt.bfloat16)
        nc.sync.dma_start(out=wt_sb[:], in_=wt[:, :])
        iota_sb = consts.tile([P, G_W], mybir.dt.bfloat16)
        nc.sync.dma_start(out=iota_sb[:], in_=iota[:, :])
        seg_sb = consts.tile([P, 2 * n_tiles], mybir.dt.bfloat16)
        nc.sync.dma_start(out=seg_sb[:], in_=seg[:, :])

        # "touch" the consts on VectorE once so later ops don't each carry
        # multiple DMA-lane semaphore waits (walrus wait-slot limit).
        dummy_a = consts.tile([P, 1], mybir.dt.bfloat16)
        nc.vector.tensor_copy(out=dummy_a[:], in_=iota_sb[:, :1])
        dummy_b = consts.tile([P, 1], mybir.dt.bfloat16)
        nc.vector.tensor_copy(out=dummy_b[:], in_=seg_sb[:, :1])
        dummy_c = consts.tile([K_DIM, 1], mybir.dt.bfloat16)
        nc.vector.tensor_copy(out=dummy_c[:], in_=wt_sb[:, :1])
        zeros_sb = consts.tile([P, P], mybir.dt.bfloat16)
        nc.vector.memset(zeros_sb[:], 0.0)
        # prewarm ScalarE's activation table during the initial x DMA
        dummy_d = consts.tile([P, 1], mybir.dt.bfloat16)
        nc.scalar.activation(
            out=dummy_d[:], in_=dummy_b[:],
            func=mybir.ActivationFunctionType.Relu,
        )

        # warm the PE HAM clock gate during the initial DMA fill: ~16
        # zero matmuls (N=512) keep the array busy ~3.4us so real work
        # starts at 2.4 GHz.  Depends only on seg/wt DMA + zeros memset.
        warm_ps = psum_h.tile([P, CHUNK * D_OUT], mybir.dt.float32, tag="hp")
        for w in range(WARM_MM):
            nc.tensor.matmul(
                out=warm_ps[:, :512],
                lhsT=zeros_sb[:K_DIM, :],
                rhs=seg_sb[:K_DIM, :512],
                start=True,
                stop=True,
                skip_group_check=True,
            )

        GH, GL = G_W // 2, 2

        def emit_oh(sw, oh_t, part, n_parts):
            """One slice of superwindow sw's one-hot (t-major layout):
            oh[p, m*G_W + g] = (seg[p, base+m] == g).  Expressed over dims
            (m, g_hi, g_lo) so every operand's innermost AP dim is a packed
            pair -> DVE 2x_1p mode.  `part` slices the tile range."""
            base_t = sw * sw_tiles
            m0 = part * sw_tiles // n_parts
            m1 = (part + 1) * sw_tiles // n_parts
            nm = m1 - m0
            o = oh_t[:, (m0 * G_W) : (m1 * G_W)]
            out_ap = bass.AP(
                tensor=o.tensor, offset=o.offset,
                ap=[o.ap[0], [G_W, nm], [GL, GH], [1, GL]],
            )
            iap = iota_sb[:]
            in0 = bass.AP(
                tensor=iap.tensor, offset=iap.offset,
                ap=[iap.ap[0], [0, nm], [GL, GH], [1, GL]],
            )
            s2 = seg_sb[:, 2 * (base_t + m0) : 2 * (base_t + m1)]
            in1 = bass.AP(
                tensor=s2.tensor, offset=s2.offset,
                ap=[s2.ap[0], [2, nm], [0, GH], [1, GL]],
            )
            nc.vector.tensor_tensor(
                out=out_ap, in0=in0, in1=in1, op=mybir.AluOpType.is_equal,
            )

        oh_tiles = {}

        def new_oh_tile(sw):
            t = ohpool.tile([P, G_W * sw_tiles], mybir.dt.bfloat16)
            oh_tiles[sw] = t
            return t

        # superwindow 0's one-hot upfront (runs during initial x DMA)
        oh0 = new_oh_tile(0)
        for part in range(2):
            emit_oh(0, oh0, part, 2)

        # Software-pipelined chunk loop: h-matmuls are emitted LOOKAHEAD
        # chunks ahead of the relu + seg-matmuls, so the in-order PE queue
        # always holds ready h work while a chunk's relu completes.  Without
        # this the PE idles each chunk (seg-MMs head-block the next h-MMs),
        # its duty cycle drops below the HAM busy threshold, and the whole
        # kernel runs at the cold 1.2 GHz clock.
        LOOKAHEAD = 2
        n_chunks = sw_tiles // CHUNK
        chunks = [(sw, chv) for sw in range(SUPER) for chv in range(n_chunks)]
        n_total = len(chunks)
        h_ctx = {}
        sw_state = {}

        def emit_h(ci):
            sw, chv = chunks[ci]
            x_pieces, piece = x_pieces_sw[sw]
            h_ps = psum_h.tile([P, CHUNK * D_OUT], mybir.dt.float32, tag="hp")
            for i in range(CHUNK):
                t = chv * CHUNK + i
                pi = (t * P) // piece
                toff = pi * piece
                nc.tensor.matmul(
                    out=h_ps[:, i * D_OUT : (i + 1) * D_OUT],
                    lhsT=x_pieces[pi][:, t * P - toff : (t + 1) * P - toff],
                    rhs=wt_sb[:],
                    start=True,
                    stop=True,
                )
            h_ctx[ci] = h_ps

        for ci in range(n_total + LOOKAHEAD):
            if ci < n_total:
                emit_h(ci)
            j = ci - LOOKAHEAD
            if j < 0:
                continue
            sw, chv = chunks[j]
            if chv == 0:
                s_ps = psum_s.tile([P, D_OUT], mybir.dt.float32)
                # open the accumulation group over the whole bank with a
                # zero matmul (clears has_written for all 128 partitions);
                # the col-group seg-matmuls accumulate with start=False
                nc.tensor.matmul(
                    out=s_ps[:],
                    lhsT=zeros_sb[:K_DIM, :],
                    rhs=wt_sb[:],
                    start=True,
                    stop=False,
                    skip_group_check=True,
                )
                oh_win = oh_tiles.pop(sw)
                oh_next = new_oh_tile(sw + 1) if sw + 1 < SUPER else None
                sw_state[sw] = (s_ps, oh_win, oh_next)
            s_ps, oh_win, oh_next = sw_state[sw]
            h_ps = h_ctx.pop(j)
            h_s = hpool_s.tile([P, RSPLIT * D_OUT], mybir.dt.bfloat16)
            nc.scalar.activation(
                out=h_s[:],
                in_=h_ps[:, : RSPLIT * D_OUT],
                func=mybir.ActivationFunctionType.Relu,
            )
            h_v = hpool_v.tile([P, (CHUNK - RSPLIT) * D_OUT],
                               mybir.dt.bfloat16)
            nc.vector.tensor_scalar_max(
                out=h_v[:], in0=h_ps[:, RSPLIT * D_OUT :], scalar1=0.0
            )
            if oh_next is not None:
                emit_oh(sw + 1, oh_next, chv, n_chunks)
            # seg-matmuls: window q of the quad accumulates on PE
            # col-group q into psum partitions [32q, 32q+32)
            for i in range(CHUNK):
                t = chv * CHUNK + i
                q = i % QUAD
                if i < RSPLIT:
                    rhs = h_s[:, i * D_OUT : (i + 1) * D_OUT]
                else:
                    rhs = h_v[:, (i - RSPLIT) * D_OUT : (i - RSPLIT + 1) * D_OUT]
                nc.tensor.matmul(
                    out=s_ps[G_W * q : G_W * (q + 1), :],
                    lhsT=oh_win[:, t * G_W : (t + 1) * G_W],
                    rhs=rhs,
                    start=False,
                    stop=(chv == n_chunks - 1 and i == CHUNK - 1),
                    tile_position=(0, G_W * q),
                    skip_group_check=True,
                )
            if chv == n_chunks - 1:
                # epilogue: max-normalize the superwindow's 128 segment rows
                s_sb = epi.tile([P, D_OUT], mybir.dt.float32)
                nc.vector.tensor_copy(out=s_sb[:], in_=s_ps[:])
                mx = epi.tile([P, 1], mybir.dt.float32)
                nc.vector.tensor_reduce(
                    out=mx[:], in_=s_sb[:], axis=mybir.AxisListType.X,
                    op=mybir.AluOpType.max,
                )
                rc = epi.tile([P, 1], mybir.dt.float32)
                nc.vector.reciprocal(out=rc[:], in_=mx[:])
                o_sb = epi.tile([P, D_OUT], mybir.dt.float32)
                nc.vector.tensor_scalar_mul(
                    out=o_sb[:], in0=s_sb[:], scalar1=rc[:]
                )
                nc.sync.dma_start(
                    out=out[sw * P : (sw + 1) * P, :], in_=o_sb[:]
                )

    if postprocess:
        _split_multi_waits(nc)
    return nc


def _split_multi_waits(nc):
    """walrus allows a single embedded sync wait per compute instruction.
    Move extra waits onto same-engine NoOps inserted just before."""
    from concourse import mybir

    n = 0
    for f in nc.m.functions:
        for blk in f.blocks:
            new_insts = []
            for inst in blk.instructions:
                si = getattr(inst, "sync_info", None)
                if si is not None and si.on_wait and len(si.on_wait) > 1:
                    extras, keep = si.on_wait[:-1], si.on_wait[-1:]
                    for wsub in extras:
                        nop = mybir.InstNoOp(
                            name=f"{inst.name}_waitnop{n}",
                            sync_info=mybir.SyncInfo(on_wait=[wsub], on_update=[]),
                            bass_nofuse=True,
                            engine=inst.engine,
                        )
                        n += 1
                        new_insts.append(nop)
                    si.on_wait = keep
                new_insts.append(inst)
            blk.instructions[:] = new_insts


def _prepare_inputs(x, w_mat, b, batch):
    """Host-side sharding/layout. Returns (in_maps, t_q)."""
    x = np.asarray(x, dtype=np.float32)
    w_mat = np.asarray(w_mat, dtype=np.float32)
    b = np.asarray(b, dtype=np.float32)
    batch = np.asarray(batch).astype(np.int64)

    # window boundaries: window j (global, 32 segs) holds atoms [wb[j], wb[j+1])
    wb = np.searchsorted(batch, np.arange(0, NUM_SEG + 1, G_W))
    counts = np.diff(wb)
    t_q = int(np.ceil(counts.max() / P))
    t_q = ((t_q + QUAD - 1) // QUAD) * QUAD  # multiple of QUAD

    sw_tiles = QUAD * t_q
    n_tiles = SUPER * sw_tiles
    a_cols = n_tiles * P

    wt = np.zeros((K_DIM, D_OUT), dtype=BF16)
    wt[:D_IN] = w_mat.T.astype(BF16)
    wt[ONES_ROW] = b.astype(BF16)
    iota = np.broadcast_to(
        np.arange(G_W, dtype=np.float32), (P, G_W)
    ).astype(BF16)

    xb = x.astype(FP8)
    n_win_per_core = SEGS_PER_CORE // G_W  # 16
    in_maps = []
    for c in range(N_CORES):
        xt_c = np.zeros((K_DIM, a_cols), dtype=FP8)
        seg_c = np.full((n_tiles, P), PAD_ID, dtype=np.float32)
        for sw in range(SUPER):
            for q in range(QUAD):
                gw = c * n_win_per_core + sw * QUAD + q  # global window id
                a0, a1 = wb[gw], wb[gw + 1]
                cnt = a1 - a0
                loc = (batch[a0:a1] - gw * G_W).astype(np.float32)
                # tile k of this window sits at interleaved slot (k*QUAD + q)
                for k in range((cnt + P - 1) // P):
                    m = sw * sw_tiles + k * QUAD + q  # global tile index
                    s0, s1 = k * P, min((k + 1) * P, cnt)
                    nseg = s1 - s0
                    col0 = m * P
                    xt_c[:D_IN, col0 : col0 + nseg] = xb[a0 + s0 : a0 + s1].T
                    xt_c[ONES_ROW, col0 : col0 + nseg] = 1.0
                    seg_c[m, :nseg] = loc[s0:s1]
        seg_c = np.ascontiguousarray(
            np.repeat(seg_c.T, 2, axis=1)
        ).astype(BF16)
        in_maps.append({"xt": xt_c, "seg": seg_c, "wt": wt, "iota": iota})
    return in_maps, t_q


def _install_ntff_hook_shim():
    """The trimmed container's antenv lacks axon_hooks; recreate it so
    run_bass_kernel_spmd(trace=True) can profile via the axon .so."""
    import types

    if "antenv.axon_hooks" in sys.modules:
        return
    try:
        from trn_agent_boot.trn_boot import _ntff_profile_via_ctypes

        hook = _ntff_profile_via_ctypes("/opt/axon/libaxon_pjrt.so")
    except Exception:
        hook = None
    mod = types.ModuleType("antenv.axon_hooks")
    mod._hook = hook
    mod.get_axon_ntff_profile_hook = lambda: mod._hook
    mod.set_axon_ntff_profile_hook = lambda h: setattr(mod, "_hook", h)
    sys.modules["antenv.axon_hooks"] = mod


def kernel(x, W, b, batch, num_segments):
    from concourse.bass_utils import run_bass_kernel_spmd

    assert int(num_segments) == NUM_SEG
    in_maps, t_q = _prepare_inputs(x, W, b, batch)

    key = (t_q, G_W, QUAD, CHUNK, RSPLIT, XBUFS, HBUFS, OHBUFS)
    if key not in _CACHE:
        _CACHE[key] = _build_graph(t_q)
    nc = _CACHE[key]

    trace = bool(int(os.environ.get("KERNEL_TRACE", "0")))
    if trace:
        _install_ntff_hook_shim()
    res = run_bass_kernel_spmd(
        nc, in_maps, core_ids=list(range(N_CORES)), trace=trace
    )
    kernel.last_result = res
    out = np.concatenate([r["out"] for r in res.results], axis=0)
    return out.astype(np.float32)


kernel.last_result = None


# revision 26
# speedup vs baseline: 1.5077x; 1.0760x over previous
"""Trainium2 Bass kernel: AtomEmbeddingAndSumLastLayer (segment_reduce).

Computes: out = normalize(relu(segment_sum(relu(x @ W.T + b), batch)))
  x [1M, 118] f32, W [64, 118], b [64], batch [1M] sorted int in [0, 4096).

Strategy (8 NeuronCores, no collectives needed):
  - Atoms are cut at segment-aligned boundaries on the host so core c owns
    exactly segments [512c, 512(c+1)); per-core outputs concatenate.
  - Host pre-transposes x to xT [119, A] fp8-e4m3 with a ones-row at 118
    (folds the bias into the matmul); atoms are grouped into 4 "superwindows"
    of 128 segments, each made of 4 windows of 32 segments whose 128-atom
    tiles are interleaved quad-wise.
  - Device, per 128-atom tile:
      h_psum[128, 64] = xT_tile.T @ WT            (TensorE, fp8 lhsT + FWL)
      h_sb = relu(h_psum) -> bf16                 (split ScalarE / VectorE)
      s_psum[32q:32q+32, 64] += oh_t.T @ h_sb     (TensorE col-group q)
    One-hot oh is built g-major per superwindow ([128, g*sw_tiles + m] =
    (seg[m] == g)) by 32 tensor_scalar(is_equal, scalar=g) ops split between
    VectorE (4x_2p mode) and GpSimd; the seg-matmul reads it through a
    strided weights AP.  PE is pre-warmed (HAM clock) with dummy matmuls
    during the initial x DMA.
    Epilogue per superwindow on [128, 64]: rowwise max, recip, scale, DMA.
"""

import os
import sys
import numpy as np

sys.path.insert(0, "/opt/trn_rl_repo")

import ml_dtypes  # noqa: E402

N_ATOMS = 1_000_000
D_IN = 118
K_DIM = 119  # 118 features + ones-row (bias) at 118
ONES_ROW = D_IN
D_OUT = 64
NUM_SEG = 4096
N_CORES = 8
SEGS_PER_CORE = NUM_SEG // N_CORES  # 512
G_W = 32  # segments per window (one PE col-group)
QUAD = 4  # windows per superwindow (= PE col-groups used)
SUPER = SEGS_PER_CORE // (G_W * QUAD)  # 4 superwindows per core
P = 128
CHUNK = 16  # tiles per compute chunk (= 4 quads; 16*64 f32 = 2 psum banks)
RSPLIT = 12  # tiles per chunk relu'd on ScalarE (rest on VectorE)
WARM_MM = 16  # dummy matmuls to warm the PE HAM clock during initial DMA
XBUFS = 12  # x-piece ring depth (4 KB/partition pieces)
HBUFS = 4
OHBUFS = 3
PAD_ID = 200.0  # local seg id for padding atoms; never matches any g

BF16 = ml_dtypes.bfloat16
FP8 = ml_dtypes.float8_e4m3

_CACHE = {}


def _build_graph(t_q: int, postprocess: bool = True):
    """Build the SPMD Bass graph for one core.

    t_q = padded tiles per window (multiple of QUAD); each superwindow has
    QUAD * t_q interleaved tiles.
    """
    import concourse.bass as bass
    import concourse.tile as tile
    from concourse import mybir
    from contextlib import ExitStack

    sw_tiles = QUAD * t_q  # tiles per superwindow
    n_tiles = SUPER * sw_tiles
    a_cols = n_tiles * P

    nc = bass.Bass(target_bir_lowering=False)

    xt = nc.declare_dram_parameter("xt", [K_DIM, a_cols], mybir.dt.float8e4, False)
    # seg ids duplicated pairwise on the host: seg2[p, 2t+j] = seg[p, t].
    # The duplication gives every operand of the one-hot is_equal a packed
    # 2-element innermost AP dim, unlocking the DVE 2x_1p mode.
    seg = nc.declare_dram_parameter("seg", [P, 2 * n_tiles], mybir.dt.bfloat16, False)
    wt = nc.declare_dram_parameter("wt", [K_DIM, D_OUT], mybir.dt.bfloat16, False)
    iota = nc.declare_dram_parameter("iota", [P, G_W], mybir.dt.bfloat16, False)
    out = nc.declare_dram_parameter(
        "out", [SEGS_PER_CORE, D_OUT], mybir.dt.float32, True
    )

    with ExitStack() as ctx:
        tc = ctx.enter_context(tile.TileContext(nc))
        consts = ctx.enter_context(tc.tile_pool(name="consts", bufs=1))
        xpool = ctx.enter_context(tc.tile_pool(name="xp", bufs=XBUFS))
        hpool_s = ctx.enter_context(tc.tile_pool(name="hps", bufs=HBUFS))
        hpool_v = ctx.enter_context(tc.tile_pool(name="hpv", bufs=HBUFS))
        ohpool = ctx.enter_context(tc.tile_pool(name="ohp", bufs=OHBUFS))
        psum_h = ctx.enter_context(tc.tile_pool(name="psh", bufs=3, space="PSUM"))
        psum_s = ctx.enter_context(tc.tile_pool(name="pss", bufs=2, space="PSUM"))
        epi = ctx.enter_context(tc.tile_pool(name="epi", bufs=2))

        # issue ALL x-piece DMAs first, on the otherwise-idle GpSimd queue:
        # the kernel is DMA-delivery-bound (~170 GB/s per-core HBM share),
        # so x must stream continuously from t=0 with nothing ahead of it.
        # Small pieces give a fast start and a short compute tail.
        N_PIECES = 8  # per superwindow
        piece = sw_tiles * P // N_PIECES
        x_pieces_sw = []
        for sw in range(SUPER):
            base_t = sw * sw_tiles
            x_pieces = []
            for pi in range(N_PIECES):
                xp_t = xpool.tile([K_DIM, piece], mybir.dt.float8e4, tag="xp8")
                p0 = base_t * P + pi * piece
                # single queue => strictly sequential piece delivery
                # (delivery rate is HBM-share-bound either way; splitting
                # across queues reorders arrivals and stalls compute)
                nc.gpsimd.dma_start(out=xp_t[:], in_=xt[:, p0 : p0 + piece])
                x_pieces.append(xp_t)
            x_pieces_sw.append((x_pieces, piece))

        wt_sb = consts.tile([K_DIM, D_OUT], mybir.dt.bfloat16)
        nc.sync.dma_start(out=wt_sb[:], in_=wt[:, :])
        iota_sb = consts.tile([P, G_W], mybir.dt.bfloat16)
        nc.sync.dma_start(out=iota_sb[:], in_=iota[:, :])
        seg_sb = consts.tile([P, 2 * n_tiles], mybir.dt.bfloat16)
        nc.sync.dma_start(out=seg_sb[:], in_=seg[:, :])

        # "touch" the consts on VectorE once so later ops don't each carry
        # multiple DMA-lane semaphore waits (walrus wait-slot limit).
        dummy_a = consts.tile([P, 1], mybir.dt.bfloat16)
        nc.vector.tensor_copy(out=dummy_a[:], in_=iota_sb[:, :1])
        dummy_b = consts.tile([P, 1], mybir.dt.bfloat16)
        nc.vector.tensor_copy(out=dummy_b[:], in_=seg_sb[:, :1])
        dummy_c = consts.tile([K_DIM, 1], mybir.dt.bfloat16)
        nc.vector.tensor_copy(out=dummy_c[:], in_=wt_sb[:, :1])
        zeros_sb = consts.tile([P, P], mybir.dt.bfloat16)
        nc.vector.memset(zeros_sb[:], 0.0)
        # prewarm ScalarE's activation table during the initial x DMA
        dummy_d = consts.tile([P, 1], mybir.dt.bfloat16)
        nc.scalar.activation(
            out=dummy_d[:], in_=dummy_b[:],
            func=mybir.ActivationFunctionType.Relu,
        )

        # warm the PE HAM clock gate during the initial DMA fill: ~16
        # zero matmuls (N=512) keep the array busy ~3.4us so real work
        # starts at 2.4 GHz.  Depends only on seg/wt DMA + zeros memset.
        warm_ps = psum_h.tile([P, CHUNK * D_OUT], mybir.dt.float32, tag="hp")
        for w in range(WARM_MM):
            nc.tensor.matmul(
                out=warm_ps[:, :512],
                lhsT=zeros_sb[:K_DIM, :],
                rhs=seg_sb[:K_DIM, :512],
                start=True,
                stop=True,
                skip_group_check=True,
            )

        GH, GL = G_W // 2, 2

        def emit_oh(sw, oh_t, part, n_parts):
            """One slice of superwindow sw's one-hot (t-major layout):
            oh[p, m*G_W + g] = (seg[p, base+m] == g).  Expressed over dims
            (m, g_hi, g_lo) so every operand's innermost AP dim is a packed
            pair -> DVE 2x_1p mode.  `part` slices the tile range."""
            base_t = sw * sw_tiles
            m0 = part * sw_tiles // n_parts
            m1 = (part + 1) * sw_tiles // n_parts
            nm = m1 - m0
            o = oh_t[:, (m0 * G_W) : (m1 * G_W)]
            out_ap = bass.AP(
                tensor=o.tensor, offset=o.offset,
                ap=[o.ap[0], [G_W, nm], [GL, GH], [1, GL]],
            )
            iap = iota_sb[:]
            in0 = bass.AP(
                tensor=iap.tensor, offset=iap.offset,
                ap=[iap.ap[0], [0, nm], [GL, GH], [1, GL]],
            )
            s2 = seg_sb[:, 2 * (base_t + m0) : 2 * (base_t + m1)]
            in1 = bass.AP(
                tensor=s2.tensor, offset=s2.offset,
                ap=[s2.ap[0], [2, nm], [0, GH], [1, GL]],
            )
            nc.vector.tensor_tensor(
                out=out_ap, in0=in0, in1=in1, op=mybir.AluOpType.is_equal,
            )

        oh_tiles = {}

        def new_oh_tile(sw):
            t = ohpool.tile([P, G_W * sw_tiles], mybir.dt.bfloat16)
            oh_tiles[sw] = t
            return t

        # superwindow 0's one-hot upfront (runs during initial x DMA)
        oh0 = new_oh_tile(0)
        for part in range(2):
            emit_oh(0, oh0, part, 2)

        # Software-pipelined chunk loop: h-matmuls are emitted LOOKAHEAD
        # chunks ahead of the relu + seg-matmuls, so the in-order PE queue
        # always holds ready h work while a chunk's relu completes.  Without
        # this the PE idles each chunk (seg-MMs head-block the next h-MMs),
        # its duty cycle drops below the HAM busy threshold, and the whole
        # kernel runs at the cold 1.2 GHz clock.
        LOOKAHEAD = 2
        n_chunks = sw_tiles // CHUNK
        chunks = [(sw, chv) for sw in range(SUPER) for chv in range(n_chunks)]
        n_total = len(chunks)
        h_ctx = {}
        sw_state = {}

        def emit_h(ci):
            sw, chv = chunks[ci]
            x_pieces, piece = x_pieces_sw[sw]
            h_ps = psum_h.tile([P, CHUNK * D_OUT], mybir.dt.float32, tag="hp")
            for i in range(CHUNK):
                t = chv * CHUNK + i
                pi = (t * P) // piece
                toff = pi * piece
                nc.tensor.matmul(
                    out=h_ps[:, i * D_OUT : (i + 1) * D_OUT],
                    lhsT=x_pieces[pi][:, t * P - toff : (t + 1) * P - toff],
                    rhs=wt_sb[:],
                    start=True,
                    stop=True,
                )
            h_ctx[ci] = h_ps

        def _emit_epilogue(sw, s_ps):
            # max-normalize the superwindow's 128 segment rows
            s_sb = epi.tile([P, D_OUT], mybir.dt.float32)
            nc.vector.tensor_copy(out=s_sb[:], in_=s_ps[:])
            mx = epi.tile([P, 1], mybir.dt.float32)
            nc.vector.tensor_reduce(
                out=mx[:], in_=s_sb[:], axis=mybir.AxisListType.X,
                op=mybir.AluOpType.max,
            )
            rc = epi.tile([P, 1], mybir.dt.float32)
            nc.vector.reciprocal(out=rc[:], in_=mx[:])
            o_sb = epi.tile([P, D_OUT], mybir.dt.float32)
            nc.vector.tensor_scalar_mul(out=o_sb[:], in0=s_sb[:], scalar1=rc[:])
            nc.sync.dma_start(out=out[sw * P : (sw + 1) * P, :], in_=o_sb[:])

        for ci in range(n_total + LOOKAHEAD):
            if ci < n_total:
                emit_h(ci)
            j = ci - LOOKAHEAD
            if j < 0:
                continue
            sw, chv = chunks[j]
            if chv == 0:
                s_ps = psum_s.tile([P, D_OUT], mybir.dt.float32)
                # open the accumulation group over the whole bank with a
                # zero matmul (clears has_written for all 128 partitions);
                # the col-group seg-matmuls accumulate with start=False
                nc.tensor.matmul(
                    out=s_ps[:],
                    lhsT=zeros_sb[:K_DIM, :],
                    rhs=wt_sb[:],
                    start=True,
                    stop=False,
                    skip_group_check=True,
                )
                oh_win = oh_tiles.pop(sw)
                oh_next = new_oh_tile(sw + 1) if sw + 1 < SUPER else None
                sw_state[sw] = (s_ps, oh_win, oh_next)
            s_ps, oh_win, oh_next = sw_state[sw]
            h_ps = h_ctx.pop(j)
            h_s = hpool_s.tile([P, RSPLIT * D_OUT], mybir.dt.bfloat16)
            nc.scalar.activation(
                out=h_s[:],
                in_=h_ps[:, : RSPLIT * D_OUT],
                func=mybir.ActivationFunctionType.Relu,
            )
            h_v = hpool_v.tile([P, (CHUNK - RSPLIT) * D_OUT],
                               mybir.dt.bfloat16)
            nc.vector.tensor_scalar_max(
                out=h_v[:], in0=h_ps[:, RSPLIT * D_OUT :], scalar1=0.0
            )
            # next superwindow's one-hot: 2 slices per chunk over the first
            # half so it's fully ready well before the boundary (a late
            # one-hot idles the PE there and cools the HAM clock)
            if oh_next is not None and chv < n_chunks // 2:
                emit_oh(sw + 1, oh_next, 2 * chv, n_chunks)
                emit_oh(sw + 1, oh_next, 2 * chv + 1, n_chunks)
            # seg-matmuls: window q of the quad accumulates on PE
            # col-group q into psum partitions [32q, 32q+32)
            for i in range(CHUNK):
                t = chv * CHUNK + i
                q = i % QUAD
                if i < RSPLIT:
                    rhs = h_s[:, i * D_OUT : (i + 1) * D_OUT]
                else:
                    rhs = h_v[:, (i - RSPLIT) * D_OUT : (i - RSPLIT + 1) * D_OUT]
                nc.tensor.matmul(
                    out=s_ps[G_W * q : G_W * (q + 1), :],
                    lhsT=oh_win[:, t * G_W : (t + 1) * G_W],
                    rhs=rhs,
                    start=False,
                    stop=(chv == n_chunks - 1 and i == CHUNK - 1),
                    tile_position=(0, G_W * q),
                    skip_group_check=True,
                )
            # epilogue for superwindow sw-1 is emitted 3 chunks INTO sw so
            # its DVE ops don't sit ahead of sw's first relu ops in the DVE
            # queue right at the boundary (the psum_s bank stays valid until
            # sw+1's opener, which waits on the copy)
            if chv == 2 and sw > 0:
                _emit_epilogue(sw - 1, sw_state[sw - 1][0])
            if chv == n_chunks - 1 and sw == SUPER - 1:
                _emit_epilogue(sw, s_ps)

    if postprocess:
        _split_multi_waits(nc)
    return nc


def _split_multi_waits(nc):
    """walrus allows a single embedded sync wait per compute instruction.
    Move extra waits onto same-engine NoOps inserted just before."""
    from concourse import mybir

    n = 0
    for f in nc.m.functions:
        for blk in f.blocks:
            new_insts = []
            for inst in blk.instructions:
                si = getattr(inst, "sync_info", None)
                if si is not None and si.on_wait and len(si.on_wait) > 1:
                    extras, keep = si.on_wait[:-1], si.on_wait[-1:]
                    for wsub in extras:
                        nop = mybir.InstNoOp(
                            name=f"{inst.name}_waitnop{n}",
                            sync_info=mybir.SyncInfo(on_wait=[wsub], on_update=[]),
                            bass_nofuse=True,
                            engine=inst.engine,
                        )
                        n += 1
                        new_insts.append(nop)
                    si.on_wait = keep
                new_insts.append(inst)
            blk.instructions[:] = new_insts


def _prepare_inputs(x, w_mat, b, batch):
    """Host-side sharding/layout. Returns (in_maps, t_q)."""
    x = np.asarray(x, dtype=np.float32)
    w_mat = np.asarray(w_mat, dtype=np.float32)
    b = np.asarray(b, dtype=np.float32)
    batch = np.asarray(batch).astype(np.int64)

    # window boundaries: window j (global, 32 segs) holds atoms [wb[j], wb[j+1])
    wb = np.searchsorted(batch, np.arange(0, NUM_SEG + 1, G_W))
    counts = np.diff(wb)
    t_q = int(np.ceil(counts.max() / P))
    t_q = ((t_q + QUAD - 1) // QUAD) * QUAD  # multiple of QUAD

    sw_tiles = QUAD * t_q
    n_tiles = SUPER * sw_tiles
    a_cols = n_tiles * P

    wt = np.zeros((K_DIM, D_OUT), dtype=BF16)
    wt[:D_IN] = w_mat.T.astype(BF16)
    wt[ONES_ROW] = b.astype(BF16)
    iota = np.broadcast_to(
        np.arange(G_W, dtype=np.float32), (P, G_W)
    ).astype(BF16)

    xb = x.astype(FP8)
    n_win_per_core = SEGS_PER_CORE // G_W  # 16
    in_maps = []
    for c in range(N_CORES):
        xt_c = np.zeros((K_DIM, a_cols), dtype=FP8)
        seg_c = np.full((n_tiles, P), PAD_ID, dtype=np.float32)
        for sw in range(SUPER):
            for q in range(QUAD):
                gw = c * n_win_per_core + sw * QUAD + q  # global window id
                a0, a1 = wb[gw], wb[gw + 1]
                cnt = a1 - a0
                loc = (batch[a0:a1] - gw * G_W).astype(np.float32)
                # tile k of this window sits at interleaved slot (k*QUAD + q)
                for k in range((cnt + P - 1) // P):
                    m = sw * sw_tiles + k * QUAD + q  # global tile index
                    s0, s1 = k * P, min((k + 1) * P, cnt)
                    nseg = s1 - s0
                    col0 = m * P
                    xt_c[:D_IN, col0 : col0 + nseg] = xb[a0 + s0 : a0 + s1].T
                    xt_c[ONES_ROW, col0 : col0 + nseg] = 1.0
                    seg_c[m, :nseg] = loc[s0:s1]
        seg_c = np.ascontiguousarray(
            np.repeat(seg_c.T, 2, axis=1)
        ).astype(BF16)
        in_maps.append({"xt": xt_c, "seg": seg_c, "wt": wt, "iota": iota})
    return in_maps, t_q


def _install_ntff_hook_shim():
    """The trimmed container's antenv lacks axon_hooks; recreate it so
    run_bass_kernel_spmd(trace=True) can profile via the axon .so."""
    import types

    if "antenv.axon_hooks" in sys.modules:
        return
    try:
        from trn_agent_boot.trn_boot import _ntff_profile_via_ctypes

        hook = _ntff_profile_via_ctypes("/opt/axon/libaxon_pjrt.so")
    except Exception:
        hook = None
    mod = types.ModuleType("antenv.axon_hooks")
    mod._hook = hook
    mod.get_axon_ntff_profile_hook = lambda: mod._hook
    mod.set_axon_ntff_profile_hook = lambda h: setattr(mod, "_hook", h)
    sys.modules["antenv.axon_hooks"] = mod


def kernel(x, W, b, batch, num_segments):
    from concourse.bass_utils import run_bass_kernel_spmd

    assert int(num_segments) == NUM_SEG
    in_maps, t_q = _prepare_inputs(x, W, b, batch)

    key = (t_q, G_W, QUAD, CHUNK, RSPLIT, XBUFS, HBUFS, OHBUFS)
    if key not in _CACHE:
        _CACHE[key] = _build_graph(t_q)
    nc = _CACHE[key]

    trace = bool(int(os.environ.get("KERNEL_TRACE", "0")))
    if trace:
        _install_ntff_hook_shim()
    res = run_bass_kernel_spmd(
        nc, in_maps, core_ids=list(range(N_CORES)), trace=trace
    )
    kernel.last_result = res
    out = np.concatenate([r["out"] for r in res.results], axis=0)
    return out.astype(np.float32)


kernel.last_result = None
